# revision 1
# baseline (speedup 1.0000x reference)
"""GNN message-passing layer on 8 TRN2 NeuronCores.

Reference computation (N=16384, D=128):
    a    = adj_mat.astype(f32)            # [N, N]
    deg  = a.sum(axis=0)                  # [N]
    agg  = (a^T @ x) / deg[:, None]       # [N, D]
    out  = relu(agg @ U^T)[None]          # [1, N, D]

Sharding: column-shard adj_mat across the 8 cores (core c owns output
nodes i in [c*2048, (c+1)*2048) and reads adj[:, islice]); x and U are
replicated. The contraction over j (all 16384 rows) is then fully local
to each core — no collective is needed, and each core emits its own
contiguous slice of the output.

Host-side staging (part of the sharding step, all value-lossless):
  - adj shard -> fp16 [16384, 2048]  ({0,1} exact; halves HBM traffic,
    which is the roofline term for this memory-bound problem)
  - x -> fp16 in [p, jb, d] layout so the device DMA is one contiguous
    transfer; U -> U^T fp16.

Per-core kernel:
  - adj shard is streamed in 128 row-blocks of [128, 2048] fp16.
  - aggT[d, i] = sum_j x[j, d] * a[j, i] accumulates in PSUM via
    matmul(lhsT=x_block [j,128d], rhs=a_block [j,512i]) — 4 psum banks
    of [128, 512] span the core's 2048 i values.
  - deg accumulates with a ones [j, 1] stationary vector. The four
    M=1 deg matmuls per block are packed into distinct 32-column PE
    groups (tile_position=(0, 32k)) so they execute concurrently on
    the 32x32 sub-arrays, costing ~1 matmul instead of 4. They share
    one PSUM bank at partitions {0, 32, 64, 96}.
  - finale: drain aggT to fp16 SBUF, transpose deg (4 x 512 rows ->
    [128, 16]) via a small DRAM bounce, reciprocal on DVE, then per
    128-i tile: h = matmul(lhsT=aggT[:, islice], rhs=U^T) -> psum
    [i, e] and relu(h * (1/deg_i)) fused into the psum->SBUF copy
    (ScalarE activation / DVE tensor_scalar, alternating). The output
    leaves as one [128, 16*128] DMA in [i_lo, t, e] layout which the
    host un-permutes.

fp16 is exact for the adjacency and deg; x/U rounding gives ~3e-4
relative error. All accumulation is fp32 in PSUM.
"""

import sys

if "/opt/trn_rl_repo" not in sys.path:
    sys.path.insert(0, "/opt/trn_rl_repo")

import numpy as np

from concourse import bacc, mybir, tile
from concourse.bass import ts
from concourse.bass_utils import run_bass_kernel_spmd

N = 16384  # nodes
D = 128  # features
CORES = 8
S = N // CORES  # 2048 output nodes per core
P = 128  # partitions
JB = N // P  # 128 row-blocks
IC = S // 512  # 4 moving-dim chunks of 512
T = S // P  # 16 output tiles per core

F16 = mybir.dt.float16
F32 = mybir.dt.float32
F8 = mybir.dt.float8e4


def build_nc():
    nc = bacc.Bacc("TRN2", target_bir_lowering=False, debug=False)

    a_dram = nc.dram_tensor("a", [N, S], F8, kind="ExternalInput").ap()
    x_dram = nc.dram_tensor("x", [P, JB * D], F16, kind="ExternalInput").ap()
    ut_dram = nc.dram_tensor("ut", [D, D], F16, kind="ExternalInput").ap()
    # [i_lo, t, e] layout; host un-permutes to [2048, 128]
    out_dram = nc.dram_tensor("out", [P, T * D], F32, kind="ExternalOutput").ap()

    with tile.TileContext(nc) as tc:
        CH = 8  # row-blocks per adj DMA chunk (2 MB fp8 transfers)
        with (
            tc.tile_pool(name="persist", bufs=1) as persist,
            tc.tile_pool(name="adj", bufs=4) as adj_pool,
            tc.tile_pool(name="dram", bufs=1, space="DRAM") as dram_pool,
        ):
            xh = persist.tile([P, JB, D], F16)
            ut16 = persist.tile([D, D], F16)
            # fp8 ones for the DoubleRow deg matmuls: [K, 2, 16] so the
            # middle (row-pair) dim has a 16-aligned element step
            ones8 = persist.tile([P, 2, 16], F8)
            nc.gpsimd.memset(ones8[:], 1.0)

            ag16 = persist.tile([P, S], F16)
            deg_sb = persist.tile([P, 512], F32)  # rows {0,32,64,96} hold deg
            degT = persist.tile([P, T], F32)
            rdeg = persist.tile([P, T], F32)

            with tc.tile_pool(name="mmps", bufs=1, space="PSUM") as mmps:
                ps_agg = [mmps.tile([P, 512], F32, name=f"ps_agg{i}") for i in range(IC)]
                ps_deg = mmps.tile([P, 512], F32, name="ps_deg")

                x_r = x_dram.rearrange("p (g jb d) -> p g jb d", g=4, jb=JB // 4)
                for ck in range(JB // CH):
                    af = adj_pool.tile([P, CH, S], F8, tag="af")
                    # alternate the two HWDGE rings; keep ring 1 (scalar)
                    # busy with the x/ut prologue during the first chunks
                    eng = nc.sync if ck % 2 == 0 else nc.scalar
                    nc_src = a_dram[ck * CH * P : (ck + 1) * CH * P, :]
                    src_r = nc_src.rearrange("(c p) i -> p c i", p=P)
                    if ck == 0:
                        # split the first chunk so the opening matmuls are
                        # not gated on a full 2 MB transfer
                        eng.dma_start(af[:, 0 : CH // 4, :], src_r[:, 0 : CH // 4, :])
                        eng.dma_start(af[:, CH // 4 :, :], src_r[:, CH // 4 :, :])
                    else:
                        eng.dma_start(af[:], src_r)
                    # x prologue in 1 MB chunks on ring 1: chunk g gates the
                    # matmuls from row-block 32*g, so only chunk 0 must land
                    # early — spreading the rest keeps the opening adj
                    # transfers from queuing behind them on the SDMA engines
                    if ck in (0, 3, 7, 11):
                        g = (0, 3, 7, 11).index(ck)
                        nc.scalar.dma_start(
                            xh[:, ts(g, JB // 4), :], x_r[:, g, :, :]
                        )
                        if ck == 0:
                            nc.scalar.dma_start(ut16[:], ut_dram[:])
                    for c in range(CH):
                        jb = ck * CH + c
                        first, last = jb == 0, jb == JB - 1
                        for ic in range(IC):
                            nc.tensor.matmul(
                                ps_agg[ic][:],
                                xh[:, jb, :],
                                af[:, c, ts(ic, 512)],
                                start=first,
                                stop=last,
                            )
                        for ic in range(IC):
                            nc.tensor.matmul(
                                ps_deg[32 * ic : 32 * ic + 1, :],
                                ones8[:, 0, 0:1],
                                af[:, c, ts(ic, 512)],
                                start=first,
                                stop=last,
                                tile_position=(0, 32 * ic),
                            )

                # drain deg rows first: the DRAM bounce + reciprocal chain
                # gates the finale (DVE requires partition step 1, so 4 ops)
                for ic in range(IC):
                    nc.vector.tensor_copy(
                        deg_sb[32 * ic : 32 * ic + 1, :],
                        ps_deg[32 * ic : 32 * ic + 1, :],
                    )
                for ic in range(IC):
                    eng = nc.vector if ic % 2 == 0 else nc.scalar
                    if ic % 2 == 0:
                        nc.vector.tensor_copy(ag16[:, ts(ic, 512)], ps_agg[ic][:])
                    else:
                        nc.scalar.copy(ag16[:, ts(ic, 512)], ps_agg[ic][:])

            # transpose deg -> [128, 16] via DRAM bounce (keep this exact
            # structure: splitting it across rings or interleaving the
            # gather per-slice triggers an NRT exec-unit crash)
            deg_dram = dram_pool.tile([IC, 512], F32)
            for ic in range(IC):
                nc.scalar.dma_start(
                    deg_dram[ic : ic + 1, :], deg_sb[32 * ic : 32 * ic + 1, :]
                )
            nc.scalar.dma_start(
                degT[:], deg_dram.rearrange("a b -> (a b)").rearrange("(t p) -> p t", p=P)
            )
            nc.vector.reciprocal(rdeg[:], degT[:])

            # two output halves in separate tiles so the first half's DMA
            # leaves while the second half is still computing
            o_halves = [
                persist.tile([P, T // 2, D], F32, name=f"o_half{h}") for h in range(2)
            ]
            out_r = out_dram.rearrange("p (t d) -> p t d", t=T)
            with tc.tile_pool(name="fps", bufs=3, space="PSUM") as fps:
                for t in range(T):
                    h_ps = fps.tile([P, D], F32, tag="h")
                    nc.tensor.matmul(
                        h_ps[:], ag16[:, ts(t, P)], ut16[:], start=True, stop=True
                    )
                    o_dst = o_halves[t // (T // 2)][:, t % (T // 2), :]
                    if t % 2 == 0:
                        # ScalarE: out = relu(h * rdeg)
                        nc.scalar.activation(
                            o_dst,
                            h_ps[:],
                            mybir.ActivationFunctionType.Relu,
                            scale=rdeg[:, t : t + 1],
                        )
                    else:
                        # DVE: out = max(h * rdeg, 0)
                        nc.vector.tensor_scalar(
                            o_dst,
                            h_ps[:],
                            rdeg[:, t : t + 1],
                            0.0,
                            mybir.AluOpType.mult,
                            mybir.AluOpType.max,
                        )
                nc.scalar.dma_start(out_r[:, 0 : T // 2, :], o_halves[0][:])
                nc.sync.dma_start(out_r[:, T // 2 : T, :], o_halves[1][:])

    nc.compile()
    return nc


_NC = None


def _get_nc():
    global _NC
    if _NC is None:
        _NC = build_nc()
    return _NC


def prep_in_maps(x, adj_mat, U):
    import ml_dtypes

    x = np.asarray(x, dtype=np.float32)
    adj_mat = np.asarray(adj_mat)
    U = np.asarray(U, dtype=np.float32)
    # x -> fp16 [p, jb, d] flattened to [128, JB*D]
    xt = np.ascontiguousarray(
        x.reshape(JB, P, D).transpose(1, 0, 2).astype(np.float16).reshape(P, JB * D)
    )
    ut = np.ascontiguousarray(U.T.astype(np.float16))
    # adjacency values are {0,1}: exact in fp8e4m3, and the int8 bit
    # patterns 0x00/0x38 can be produced by a table lookup (much faster
    # than a float astype over 1 GiB)
    lut = np.zeros(2, dtype=np.uint8)
    lut[1] = np.array(1.0, dtype=ml_dtypes.float8_e4m3).view(np.uint8)
    in_maps = []
    for c in range(CORES):
        a8 = lut[adj_mat[:, c * S : (c + 1) * S]].view(ml_dtypes.float8_e4m3)
        in_maps.append({"a": a8, "x": xt, "ut": ut})
    return in_maps


def assemble_out(results):
    # per-core out is [128, T*D] in [i_lo, t, e] layout
    parts = []
    for c in range(CORES):
        o = results[c]["out"].reshape(P, T, D).transpose(1, 0, 2).reshape(S, D)
        parts.append(o)
    return np.concatenate(parts, axis=0)[None]


def kernel(x, adj_mat, U, **_):
    nc = _get_nc()
    in_maps = prep_in_maps(x, adj_mat, U)
    res = run_bass_kernel_spmd(nc, in_maps, core_ids=list(range(CORES)))
    return assemble_out(res.results)



# revision 13
# speedup vs baseline: 1.3883x; 1.3883x over previous
"""GNN message-passing layer on 8 TRN2 NeuronCores.

Reference computation (N=16384, D=128):
    a    = adj_mat.astype(f32)            # [N, N]
    deg  = a.sum(axis=0)                  # [N]
    agg  = (a^T @ x) / deg[:, None]       # [N, D]
    out  = relu(agg @ U^T)[None]          # [1, N, D]

Sharding: column-shard adj_mat across the 8 cores (core c owns output
nodes i in [c*2048, (c+1)*2048) and reads adj[:, islice]); x and U are
replicated. The contraction over j (all 16384 rows) is fully local to
each core — no collective — and each core emits its own contiguous
slice of the output.

PE scheme (the kernel is tensor-engine bound): the aggregation runs as
fp8e4 DoubleRow matmuls — 256 contraction rows per pass, 2x the
fp16/bf16 rate.  x is split hi/lo (x = fp8(x) + fp8(x - fp8(x)), ~8
mantissa bits combined) so precision matches fp16, and the two passes
per 256-row double-block cost exactly what one fp16 pass over the same
rows would.  The win over the fp16 baseline: deg rides inside the
weight matrices instead of costing its own matmuls —
  pass A weights = [ones | x_hi[:, 1:]]      -> psum row 0 = deg
  pass B weights = [x_hi[:, 0] | x_lo[:, 1:]] -> psum row 0 = agg_0
Feature 0 therefore gets only single-fp8 precision; its error
contributes ~2.5e-2/sqrt(128) ~ 2e-3 overall, well inside the 2e-2
gate (measured end-to-end ~2.2e-3).  The drain sums A+B on all 128
rows (row 0 of the sum is garbage) and then overwrites row 0 with
B's row 0 — all engine accesses stay 32-partition aligned, which the
BIR verifier requires.

Host-side staging (value-lossless, part of sharding): the adjacency is
row-permuted per 1 MiB chunk so every device DMA is 128 partitions x
8 KiB contiguous (descriptor-cheap), and converted {0,1}->fp8e4 via a
uint8 LUT.  x -> xA/xB fp8 weight tensors in [p, pair, ktile, col]
layout; U -> U^T fp16.
"""

import sys

if "/opt/trn_rl_repo" not in sys.path:
    sys.path.insert(0, "/opt/trn_rl_repo")

import numpy as np

from concourse import bacc, mybir, tile
from concourse.bass import ts
from concourse.bass_utils import run_bass_kernel_spmd

N = 16384  # nodes
D = 128  # features
CORES = 8
S = N // CORES  # 2048 output nodes per core
P = 128  # partitions
JB = N // P  # 128 row-blocks
NPAIR = JB // 2  # 64 DoubleRow double-blocks
IC = S // 512  # 4 psum chunks of 512
T = S // P  # 16 output tiles per core
CH = 8  # row-blocks per adj DMA chunk (2 MB fp8)
NCK = JB // CH  # 16 chunks

F16 = mybir.dt.float16
F32 = mybir.dt.float32
F8 = mybir.dt.float8e4
DR = mybir.MatmulPerfMode.DoubleRow


def build_nc():
    nc = bacc.Bacc("TRN2", target_bir_lowering=False, debug=False)

    a_dram = nc.dram_tensor("a", [N, S], F8, kind="ExternalInput").ap()
    xa_dram = nc.dram_tensor("xa", [P, NPAIR * 2 * D], F8, kind="ExternalInput").ap()
    xb_dram = nc.dram_tensor("xb", [P, NPAIR * 2 * D], F8, kind="ExternalInput").ap()
    ut_dram = nc.dram_tensor("ut", [D, D], F16, kind="ExternalInput").ap()
    # [i_lo, t, e] layout; host un-permutes to [2048, 128]
    out_dram = nc.dram_tensor("out", [P, T * D], F32, kind="ExternalOutput").ap()

    with tile.TileContext(nc) as tc:
        with (
            tc.tile_pool(name="persist", bufs=1) as persist,
            tc.tile_pool(name="adj", bufs=4) as adj_pool,
            tc.tile_pool(name="dram", bufs=1, space="DRAM") as dram_pool,
        ):
            xa = persist.tile([P, NPAIR, 2, D], F8)
            xb = persist.tile([P, NPAIR, 2, D], F8)
            ut16 = persist.tile([D, D], F16)

            ag16 = persist.tile([P, S], F16)
            bsb = persist.tile([P, S], F32)  # psum-B staging (single-PSUM-input rule)
            deg_sb = persist.tile([P, 512], F32)  # rows {0,32,64,96} hold deg
            degT = persist.tile([P, T], F32)
            rdeg = persist.tile([P, T], F32)

            with tc.tile_pool(name="mmps", bufs=1, space="PSUM") as mmps:
                ps_a = [mmps.tile([P, 512], F32, name=f"ps_a{i}") for i in range(IC)]
                ps_b = [mmps.tile([P, 512], F32, name=f"ps_b{i}") for i in range(IC)]

                xa_r = xa_dram.rearrange("p (g r) -> p g r", g=8)
                xb_r = xb_dram.rearrange("p (g r) -> p g r", g=8)
                for ck in range(NCK):
                    af = adj_pool.tile([P, CH, S], F8, tag="af")
                    # alternate the two HWDGE rings; ring 1 (scalar) also
                    # carries the x prologue
                    eng = nc.sync if ck % 2 == 0 else nc.scalar
                    # host layout: chunk ck rows are [p, c, i] with the
                    # per-partition CH*S bytes contiguous
                    src = a_dram[ck * CH * P : (ck + 1) * CH * P, :]
                    src_r = src.rearrange("(p c) i -> p c i", p=P)
                    if ck == 0:
                        # split the first chunk so the opening matmuls are
                        # not gated on a full 2 MB transfer
                        eng.dma_start(af[:, 0:2, :], src_r[:, 0:2, :])
                        eng.dma_start(af[:, 2:4, :], src_r[:, 2:4, :])
                        eng.dma_start(af[:, 4:CH, :], src_r[:, 4:CH, :])
                    else:
                        eng.dma_start(af[:], src_r)
                    # x weights prologue in 512 KB pieces on ring 1: piece g
                    # gates the matmuls from pair 8g
                    if ck in (0, 1, 3, 5, 7, 9, 11, 13):
                        g = (0, 1, 3, 5, 7, 9, 11, 13).index(ck)
                        nc.scalar.dma_start(xa[:, ts(g, 8), :, :], xa_r[:, g, :])
                        nc.scalar.dma_start(xb[:, ts(g, 8), :, :], xb_r[:, g, :])
                        if ck == 0:
                            nc.scalar.dma_start(ut16[:], ut_dram[:])
                    for pp in range(CH // 2):
                        b = ck * (CH // 2) + pp
                        first, last = b == 0, b == NPAIR - 1
                        mv = af[:, 2 * pp : 2 * pp + 2, :]
                        for ic in range(IC):
                            nc.tensor.matmul(
                                ps_a[ic][:],
                                xa[:, b, :, :],
                                mv[:, :, ts(ic, 512)],
                                start=first,
                                stop=last,
                                perf_mode=DR,
                            )
                        for ic in range(IC):
                            nc.tensor.matmul(
                                ps_b[ic][:],
                                xb[:, b, :, :],
                                mv[:, :, ts(ic, 512)],
                                start=first,
                                stop=last,
                                perf_mode=DR,
                            )

                # deg rows out first: they gate the DRAM-bounce + reciprocal
                # chain that the finale scale depends on
                for ic in range(IC):
                    nc.vector.tensor_copy(
                        deg_sb[32 * ic : 32 * ic + 1, :], ps_a[ic][0:1, :]
                    )
                # drain: stage B in SBUF (an instruction may read only one
                # PSUM input), then ag16 = A(psum) + B(sbuf).  Row 0 of the
                # sum is deg + agg_0 garbage; overwrite it with B's row 0
                # (the fp8-single feature-0 agg).
                for ic in range(IC):
                    nc.scalar.copy(bsb[:, ts(ic, 512)], ps_b[ic][:])
                for ic in range(IC):
                    nc.vector.tensor_tensor(
                        ag16[:, ts(ic, 512)],
                        ps_a[ic][:],
                        bsb[:, ts(ic, 512)],
                        mybir.AluOpType.add,
                    )
                for ic in range(IC):
                    nc.gpsimd.tensor_copy(ag16[0:1, ts(ic, 512)], bsb[0:1, ts(ic, 512)])

            # transpose deg -> [128, 16] via DRAM bounce (keep this exact
            # structure: splitting it across rings or interleaving the
            # gather per-slice triggers an NRT exec-unit crash)
            deg_dram = dram_pool.tile([IC, 512], F32)
            for ic in range(IC):
                nc.scalar.dma_start(
                    deg_dram[ic : ic + 1, :], deg_sb[32 * ic : 32 * ic + 1, :]
                )
            nc.scalar.dma_start(
                degT[:], deg_dram.rearrange("a b -> (a b)").rearrange("(t p) -> p t", p=P)
            )
            nc.vector.reciprocal(rdeg[:], degT[:])

            # two output halves in separate tiles so the first half's DMA
            # leaves while the second half is still computing
            o_halves = [
                persist.tile([P, T // 2, D], F32, name=f"o_half{h}") for h in range(2)
            ]
            out_r = out_dram.rearrange("p (t d) -> p t d", t=T)
            with tc.tile_pool(name="fps", bufs=3, space="PSUM") as fps:
                for t in range(T):
                    h_ps = fps.tile([P, D], F32, tag="h")
                    nc.tensor.matmul(
                        h_ps[:], ag16[:, ts(t, P)], ut16[:], start=True, stop=True
                    )
                    o_dst = o_halves[t // (T // 2)][:, t % (T // 2), :]
                    if t % 2 == 0:
                        # ScalarE: out = relu(h * rdeg)
                        nc.scalar.activation(
                            o_dst,
                            h_ps[:],
                            mybir.ActivationFunctionType.Relu,
                            scale=rdeg[:, t : t + 1],
                        )
                    else:
                        # DVE: out = max(h * rdeg, 0)
                        nc.vector.tensor_scalar(
                            o_dst,
                            h_ps[:],
                            rdeg[:, t : t + 1],
                            0.0,
                            mybir.AluOpType.mult,
                            mybir.AluOpType.max,
                        )
                nc.scalar.dma_start(out_r[:, 0 : T // 2, :], o_halves[0][:])
                nc.sync.dma_start(out_r[:, T // 2 : T, :], o_halves[1][:])

    nc.compile()
    return nc


_NC = None


def _get_nc():
    global _NC
    if _NC is None:
        _NC = build_nc()
    return _NC


# adjacency row permutation: DMA chunk ck wants rows in [p, c] order so
# each partition's CH rows are contiguous in DRAM
def _adj_perm():
    idx = np.arange(N).reshape(NCK, CH, P)  # [ck, c, p]
    return idx.transpose(0, 2, 1).reshape(-1)  # [ck, p, c]


def prep_in_maps(x, adj_mat, U):
    import ml_dtypes

    f8 = ml_dtypes.float8_e4m3
    x = np.asarray(x, dtype=np.float32)
    adj_mat = np.asarray(adj_mat)
    U = np.asarray(U, dtype=np.float32)

    xhi = x.astype(f8).astype(np.float32)
    xlo = (x - xhi).astype(f8).astype(np.float32)
    # weight tensors [P, NPAIR, 2, D]: row j = pair*256 + ktile*128 + p.
    # column 0 carries deg (pass A: ones) and the fp8-single feature 0
    # (pass B); columns 1.. carry hi/lo of features 1..
    wA = np.concatenate([np.ones((N, 1), np.float32), xhi[:, 1:]], axis=1)
    wB = np.concatenate([xhi[:, 0:1], xlo[:, 1:]], axis=1)

    def wfmt(w):
        # [N, D] -> [pair, ktile, p, d] -> [p, pair, ktile, d]
        v = w.reshape(NPAIR, 2, P, D).transpose(2, 0, 1, 3)
        return np.ascontiguousarray(v.astype(f8).reshape(P, NPAIR * 2 * D))

    xa = wfmt(wA)
    xb = wfmt(wB)
    ut = np.ascontiguousarray(U.T.astype(np.float16))

    # adjacency values are {0,1}: exact in fp8e4m3; the int8 bit patterns
    # 0x00/0x38 come from a uint8 LUT (much faster than a float astype
    # over 1 GiB).  Rows are permuted so each DMA chunk is contiguous
    # per partition.
    lut = np.zeros(2, dtype=np.uint8)
    lut[1] = np.array(1.0, dtype=f8).view(np.uint8)
    perm = _adj_perm()
    adj_p = adj_mat[perm]
    in_maps = []
    for c in range(CORES):
        a8 = lut[adj_p[:, c * S : (c + 1) * S]].view(f8)
        in_maps.append({"a": a8, "xa": xa, "xb": xb, "ut": ut})
    return in_maps


def assemble_out(results):
    # per-core out is [128, T*D] in [i_lo, t, e] layout
    parts = []
    for c in range(CORES):
        o = results[c]["out"].reshape(P, T, D).transpose(1, 0, 2).reshape(S, D)
        parts.append(o)
    return np.concatenate(parts, axis=0)[None]


def kernel(x, adj_mat, U, **_):
    nc = _get_nc()
    in_maps = prep_in_maps(x, adj_mat, U)
    res = run_bass_kernel_spmd(nc, in_maps, core_ids=list(range(CORES)))
    return assemble_out(res.results)
